# revision 16
# baseline (speedup 1.0000x reference)
"""Sparse attention (GTP-ViT style) Trainium2 Bass kernel.

Shapes (hardcoded): B=8, N=1024, C=768, H=12, hd=64, SPARSITY=0.5.
Sharding: data-parallel over batch — core b computes batch b entirely
(qkv proj, per-head attention + kth-value sparsification, out proj).

The kth-value threshold (median of the 1M softmax probs per (b,h)) is
exact: the same NEFF runs twice. Run 1 uses threshold=-inf, producing
the dense softmax attn; the host takes the exact k-th smallest value
per (b,h) via np.partition; run 2 applies the real thresholds and
produces the final sparse attn + projected output. One compile, and
the thresholds are self-consistent with the device-computed probs.

Device math per core (batch b), all fp32:
  qkvT[j,n]  = sum_c w_qkvT[c,j]*xT[c,n] (+bias, q rows pre-scaled by hd^-0.5)
  S[n,m]     = qT.T @ kT  +  ones.T @ log(ts)   (rank-1 accumulated matmul)
  pex        = exp(S)            (ACT, with fused row-sum accum_out)
  attn       = pex * ((pex >= t*densum) * (1/densum))   (2 DVE passes)
  OT[d,n]    = sum_m v[m,d]*attnT[m,n]   (attnT via PE transposes)
  outT[c',n] = w_projT.T @ OT (+bias)
"""

import os
import sys

import numpy as np

sys.path.insert(0, "/opt/trn_rl_repo")

import concourse.bass as bass
import concourse.bacc as bacc
import concourse.mybir as mybir
from concourse import tile
from concourse.bass_utils import run_bass_kernel_spmd

FP32 = mybir.dt.float32

B, N, C = 8, 1024, 768
H, HD = 12, 64
NCORES = 8
KTH = int(N * N * 0.5)  # 524288
SCALE = HD ** (-0.5)

# float32r: PE runs 1 cyc/row (vs 4 for fp32); producers round on write.
FP32R = mybir.dt.float32r


def build_nc():
    nc = bacc.Bacc(
        "TRN2",
        target_bir_lowering=False,
        debug=False,
        num_devices=NCORES,
    )

    xT = nc.declare_dram_parameter("xT", [C, N], FP32R, isOutput=False)
    wqkvT = nc.declare_dram_parameter("wqkvT", [C, 3 * C], FP32R, isOutput=False)
    bqkv = nc.declare_dram_parameter("bqkv", [3 * C], FP32, isOutput=False)
    wprojT = nc.declare_dram_parameter("wprojT", [C, C], FP32R, isOutput=False)
    bproj = nc.declare_dram_parameter("bproj", [C], FP32, isOutput=False)
    ts = nc.declare_dram_parameter("ts", [1, N], FP32, isOutput=False)
    thr = nc.declare_dram_parameter("thr", [H, 128], FP32, isOutput=False)
    ident = nc.declare_dram_parameter("ident", [128, 128], FP32, isOutput=False)

    attn_o = nc.declare_dram_parameter("attn_o", [H, N, N], FP32, isOutput=True)
    outT_o = nc.declare_dram_parameter("outT_o", [C, N], FP32, isOutput=True)

    CC = C // 128      # 6 chunks of the contraction dim
    JT = 3 * C // 128  # 18 row-tiles of qkvT
    NT = N // 128      # 8 row-tiles of the attn matrix

    with tile.TileContext(nc) as tc:
        with (
            tc.tile_pool(name="consts", bufs=1) as cst,
            tc.tile_pool(name="qkv", bufs=1) as qkp,
            tc.tile_pool(name="psA", bufs=2, space=bass.MemorySpace.PSUM) as psA,
            tc.tile_pool(name="psB", bufs=2, space=bass.MemorySpace.PSUM) as psB,
            tc.tile_pool(name="psC", bufs=2, space=bass.MemorySpace.PSUM) as psC,
        ):
            # ---- constants ----
            bqkv_s = cst.tile([128, JT], FP32, tag="bqkv")
            nc.sync.dma_start(bqkv_s[:], bqkv.rearrange("(jt p) -> p jt", p=128))
            wprojT_s = cst.tile([128, CC, C], FP32R, tag="wprojT")
            nc.sync.dma_start(wprojT_s[:], wprojT.rearrange("(cc p) j -> p cc j", p=128))
            bproj_s = cst.tile([128, CC], FP32, tag="bproj")
            nc.sync.dma_start(bproj_s[:], bproj.rearrange("(ct p) -> p ct", p=128))
            thr_s = cst.tile([128, H], FP32, tag="thr")
            nc.sync.dma_start(thr_s[:], thr.rearrange("h p -> p h"))
            ident_s = cst.tile([128, 128], FP32, tag="ident")
            nc.sync.dma_start(ident_s[:], ident[:])
            ts_s = cst.tile([1, N], FP32, tag="ts")
            nc.sync.dma_start(ts_s[:], ts[:])
            logts = cst.tile([1, N], FP32R, tag="logts")
            nc.scalar.activation(logts[:], ts_s[:], mybir.ActivationFunctionType.Ln)
            ones1f = cst.tile([1, 128], FP32, tag="ones1f")
            nc.gpsimd.memset(ones1f[:], 1.0)
            ones1 = cst.tile([1, 128], FP32R, tag="ones1")
            nc.scalar.copy(ones1[:], ones1f[:])
            identr = cst.tile([128, 128], FP32R, tag="identr")
            nc.scalar.copy(identr[:], ident_s[:])

            qkvT_s = qkp.tile([128, JT, N], FP32R, tag="qkvT")

            # ---- phase 1: qkvT = (x @ w_qkv.T + b).T, q rows scaled ----
            with tc.tile_pool(name="phase1", bufs=1) as p1:
                xT_s = p1.tile([128, CC, N], FP32R, tag="xT")
                nc.sync.dma_start(xT_s[:], xT.rearrange("(cc p) n -> p cc n", p=128))
                wqkvT_s = p1.tile([128, CC, 3 * C], FP32R, tag="wqkvT")
                nc.sync.dma_start(
                    wqkvT_s[:], wqkvT.rearrange("(cc p) j -> p cc j", p=128)
                )
                for jt in range(JT):
                    sc = SCALE if jt < CC else 1.0
                    for nh in range(2):
                        ps = psC.tile([128, 512], FP32, tag="mm512")
                        for ci in range(CC):
                            nc.tensor.matmul(
                                ps[:],
                                wqkvT_s[:, ci, jt * 128:(jt + 1) * 128],
                                xT_s[:, ci, nh * 512:(nh + 1) * 512],
                                start=(ci == 0),
                                stop=(ci == CC - 1),
                            )
                        nc.scalar.activation(
                            qkvT_s[:, jt, nh * 512:(nh + 1) * 512], ps[:],
                            mybir.ActivationFunctionType.Identity,
                            bias=bqkv_s[:, jt:jt + 1], scale=sc,
                        )

            # ---- phase 2: per-head attention ----
            per_cm = tc.tile_pool(name="persist", bufs=1)
            wrk_cm = tc.tile_pool(name="work", bufs=3)
            wrk2_cm = tc.tile_pool(name="work2", bufs=2)
            per, wrk, wrk2 = per_cm.__enter__(), wrk_cm.__enter__(), wrk2_cm.__enter__()
            ptb = per.tile([128, NT, N], FP32R, tag="ptb")      # attnT, one head at a time
            otb = per.tile([128, CC, N], FP32R, tag="otb")      # OT = (attn@v).T, all heads
            vbuf = per.tile([128, NT, 128], FP32R, tag="vbuf")   # v[m,d], one head at a time
            for g in range(NT):
                nc.scalar.activation(
                    vbuf[:, g, :], ident_s[:],
                    mybir.ActivationFunctionType.Copy, bias=0.0, scale=0.0,
                )
            for h in range(H):
                po = (h % 2) * 64  # partition offset of this head's 64 dims
                qT = qkvT_s[po:po + 64, h // 2, :]
                kT = qkvT_s[po:po + 64, 6 + h // 2, :]
                vT = qkvT_s[po:po + 64, 12 + h // 2, :]

                # v[m,d] from vT via PE transposes
                for g in range(NT):
                    vps = psB.tile([128, 128], FP32R, tag="t1")
                    nc.tensor.transpose(
                        vps[:, 0:64], vT[:, g * 128:(g + 1) * 128],
                        identr[po:po + 64, po:po + 64],
                    )
                    nc.scalar.copy(vbuf[:, g, po:po + 64], vps[:, 0:64])

                for nt in range(NT):
                    sps = psA.tile([128, N], FP32, tag="s2")
                    pe = wrk.tile([128, N], FP32, tag="pex")
                    den = wrk2.tile([128, 2], FP32, tag="den")
                    for mh in range(2):
                        sl = slice(mh * 512, (mh + 1) * 512)
                        nc.tensor.matmul(
                            sps[:, sl],
                            qT[:, nt * 128:(nt + 1) * 128],
                            kT[:, sl],
                            start=True, stop=False,
                        )
                        nc.tensor.matmul(
                            sps[:, sl], ones1[:], logts[:, sl],
                            start=False, stop=True,
                        )
                        nc.scalar.activation(
                            pe[:, sl], sps[:, sl],
                            mybir.ActivationFunctionType.Exp,
                            accum_out=den[:, mh:mh + 1],
                        )

                    densum = wrk2.tile([128, 1], FP32, tag="densum")
                    nc.vector.tensor_tensor(
                        densum[:], den[:, 0:1], den[:, 1:2], mybir.AluOpType.add
                    )
                    rcp = wrk2.tile([128, 1], FP32, tag="rcp")
                    nc.vector.reciprocal(rcp[:], densum[:])
                    thr2 = wrk2.tile([128, 1], FP32, tag="thr2")
                    nc.vector.tensor_scalar(
                        thr2[:], densum[:], thr_s[:, h:h + 1], None,
                        mybir.AluOpType.mult,
                    )

                    mr = wrk2.tile([128, N], FP32, tag="mr")
                    # (pex >= t*densum) * (1/densum)
                    nc.vector.tensor_scalar(
                        mr[:], pe[:], thr2[:], rcp[:],
                        mybir.AluOpType.is_ge, mybir.AluOpType.mult,
                    )
                    att = wrk2.tile([128, N], FP32, tag="att")
                    nc.vector.tensor_tensor(att[:], pe[:], mr[:], mybir.AluOpType.mult)
                    nc.sync.dma_start(attn_o[h, nt * 128:(nt + 1) * 128, :], att[:])
                    for mt in range(NT):
                        tps = psB.tile([128, 128], FP32, tag="t1")
                        nc.tensor.transpose(
                            tps[:], att[:, mt * 128:(mt + 1) * 128], ident_s[:]
                        )
                        nc.scalar.copy(ptb[:, mt, nt * 128:(nt + 1) * 128], tps[:])

                # OT[d, n] = sum_m v[m, d] * attnT[m, n]
                for nh in range(2):
                    ops = psC.tile([128, 512], FP32, tag="mm512")
                    for mt in range(NT):
                        nc.tensor.matmul(
                            ops[:],
                            vbuf[:, mt, :],
                            ptb[:, mt, nh * 512:(nh + 1) * 512],
                            start=(mt == 0), stop=(mt == NT - 1),
                        )
                    nc.scalar.copy(
                        otb[po:po + 64, h // 2, nh * 512:(nh + 1) * 512], ops[po:po + 64, :]
                    )

            # ---- phase 3: outT = w_projT.T @ OT + b ----
            for ct in range(CC):
                for nh in range(2):
                    ps = psC.tile([128, 512], FP32, tag="mm512")
                    for ci in range(CC):
                        nc.tensor.matmul(
                            ps[:],
                            wprojT_s[:, ci, ct * 128:(ct + 1) * 128],
                            otb[:, ci, nh * 512:(nh + 1) * 512],
                            start=(ci == 0), stop=(ci == CC - 1),
                        )
                    ot = wrk.tile([128, 512], FP32, tag="outT")
                    nc.scalar.activation(
                        ot[:], ps[:], mybir.ActivationFunctionType.Identity,
                        bias=bproj_s[:, ct:ct + 1],
                    )
                    nc.sync.dma_start(
                        outT_o[ct * 128:(ct + 1) * 128, nh * 512:(nh + 1) * 512], ot[:]
                    )

            for cm in (wrk2_cm, wrk_cm, per_cm):
                cm.__exit__(None, None, None)

    nc.compile()
    return nc


_NC = None


def _get_nc():
    global _NC
    if _NC is None:
        _NC = build_nc()
    return _NC


def _in_maps(x, token_scales, w_qkv, b_qkv, w_proj, b_proj, thr):
    """Per-core input maps. thr: [NCORES, H] float32."""
    bq = b_qkv.astype(np.float32).copy()
    bq[:C] *= SCALE  # q bias pre-scaled (zero in practice)
    wqkvT = np.ascontiguousarray(w_qkv.astype(np.float32).T)
    wprojT = np.ascontiguousarray(w_proj.astype(np.float32).T)
    ident = np.eye(128, dtype=np.float32)
    maps = []
    for c in range(NCORES):
        maps.append({
            "xT": np.ascontiguousarray(x[c].astype(np.float32).T),
            "wqkvT": wqkvT,
            "bqkv": bq,
            "wprojT": wprojT,
            "bproj": b_proj.astype(np.float32),
            "ts": token_scales[c].astype(np.float32).reshape(1, N),
            "thr": np.repeat(thr[c].astype(np.float32)[:, None], 128, axis=1),
            "ident": ident,
        })
    return maps


_LAST_EXEC_NS = []


def _run(nc, maps, trace=False):
    global _LAST_EXEC_NS
    res = run_bass_kernel_spmd(nc, maps, list(range(NCORES)), trace=trace)
    if res.exec_time_ns is not None:
        _LAST_EXEC_NS.append(res.exec_time_ns)
    return res.results


def kernel(x, token_scales, w_qkv, b_qkv, w_proj, b_proj):
    x = np.asarray(x)
    token_scales = np.asarray(token_scales)
    w_qkv = np.asarray(w_qkv)
    b_qkv = np.asarray(b_qkv)
    w_proj = np.asarray(w_proj)
    b_proj = np.asarray(b_proj)

    nc = _get_nc()

    # run 1: threshold = -inf -> dense softmax attn
    thr0 = np.full((NCORES, H), -1e30, dtype=np.float32)
    res1 = _run(nc, _in_maps(x, token_scales, w_qkv, b_qkv, w_proj, b_proj, thr0))

    # exact kth smallest of each (b, h) slice
    thr = np.empty((NCORES, H), dtype=np.float32)
    for c in range(NCORES):
        dense = np.asarray(res1[c]["attn_o"]).reshape(H, N * N)
        part = np.partition(dense, KTH - 1, axis=1)
        thr[c] = part[:, KTH - 1]

    # run 2: real thresholds -> sparse attn + out
    res2 = _run(nc, _in_maps(x, token_scales, w_qkv, b_qkv, w_proj, b_proj, thr))

    attn = np.stack([np.asarray(res2[c]["attn_o"]) for c in range(NCORES)])
    out = np.stack(
        [np.ascontiguousarray(np.asarray(res2[c]["outT_o"]).T) for c in range(NCORES)]
    )
    return out, attn
